# revision 1
# baseline (speedup 1.0000x reference)
"""Trainium2 Bass kernel for a sparse-attention EncoderLayer.

Sharding: rows (L) are split into 8 contiguous shards of L/8; each edge is
owned by the core that owns its destination row (row_index is sorted, so each
core's edges are a contiguous range).  Each core computes Q/K/V for its row
shard, the K/V shards are AllGathered (bf16) so every core holds the full
K/V table in HBM, and per-edge K/V rows are fetched with dma_gather.  The
segment softmax is computed without the max-subtraction (scores here are
bounded by ~|q||k|/8 + |bias| < 10, so exp() cannot overflow in f32 and
alpha = exp(s - m)/sum exp(s - m) == exp(s)/sum exp(s)).  The alpha-weighted
scatter and the per-row softmax sums are evaluated as one-hot PE matmuls over
128-edge tiles, accumulated in PSUM per 128-row block.
"""

import math
import numpy as np
from contextlib import ExitStack

from ml_dtypes import bfloat16

import concourse.bass as bass
import concourse.mybir as mybir
import concourse.tile as tile
from concourse import bacc
from concourse.bass_utils import run_bass_kernel_spmd
from concourse.masks import make_identity

NCORES = 8
C, H, D, HID = 512, 8, 64, 1024
EPS = 1e-5
CHUNK_T = 16  # edge tiles (of 128 edges) per dma_gather chunk
F32 = mybir.dt.float32
BF16 = mybir.dt.bfloat16
I16 = mybir.dt.int16
AF = mybir.ActivationFunctionType
ALU = mybir.AluOpType
AX = mybir.AxisListType

_prog_cache = {}
TRACE = False          # set True (with the ntff hook registered) to profile
LAST_EXEC_NS = None    # exec time of the last run when TRACE was on
LAST_RESULTS = None    # full BassKernelResults of the last run


# --------------------------------------------------------------------------
# host-side preprocessing
# --------------------------------------------------------------------------

def _wrap_idx(idx):
    """[n] int -> [128, n//16] int16, wrapped (idx i at partition i%16,
    column i//16) and replicated across the 8 Q7 cores."""
    n = idx.shape[0]
    w = np.ascontiguousarray(idx.reshape(n // 16, 16).T).astype(np.int16)
    return np.tile(w, (8, 1))


def _preprocess_edges(L, row, col, att_bias):
    LSH = L // NCORES
    NBLK = LSH // 128
    bounds = np.searchsorted(row, np.arange(NCORES + 1) * LSH)

    per_core = []
    t_blk = 1
    for c in range(NCORES):
        e0, e1 = int(bounds[c]), int(bounds[c + 1])
        r = row[e0:e1] - c * LSH
        blk = r >> 7
        cnt = np.bincount(blk, minlength=NBLK)
        t_blk = max(t_blk, int(np.max((cnt + 127) // 128)) if len(cnt) else 1)
        per_core.append((e0, e1, r, blk, cnt))

    T_BLK = t_blk
    NT = NBLK * T_BLK
    NCH = (NT + CHUNK_T - 1) // CHUNK_T
    NTP = NCH * CHUNK_T  # tiles padded to whole chunks (extra tiles unused)

    cores = []
    for c in range(NCORES):
        e0, e1, r, blk, cnt = per_core[c]
        ne = e1 - e0
        starts = np.zeros(NBLK, dtype=np.int64)
        np.cumsum(cnt[:-1], out=starts[1:])
        idx_in_blk = np.arange(ne, dtype=np.int64) - starts[blk]
        dst = blk * (T_BLK * 128) + idx_in_blk

        npad = NTP * 128
        colP = np.zeros(npad, dtype=np.int64)
        qlocP = np.zeros(npad, dtype=np.int64)
        rlocP = np.zeros(npad, dtype=np.float32)
        biasP = np.full((npad, H), -30000.0, dtype=np.float32)
        colP[dst] = col[e0:e1]
        qlocP[dst] = r
        rlocP[dst] = (r & 127).astype(np.float32)
        biasP[dst] = att_bias[e0:e1]

        colw = _wrap_idx(colP).reshape(128, NCH, CHUNK_T * 8).transpose(1, 0, 2)
        qlocw = _wrap_idx(qlocP).reshape(128, NCH, CHUNK_T * 8).transpose(1, 0, 2)
        colw = colw.reshape(NCH * 128, CHUNK_T * 8)
        qlocw = qlocw.reshape(NCH * 128, CHUNK_T * 8)
        # [NT, 128, H] / [NT, 128] partition-major per tile
        biasT = biasP.reshape(NTP, 128, H)[:NT]
        rlocT = rlocP.reshape(NTP, 128)[:NT]
        cores.append(dict(
            colw=np.ascontiguousarray(colw),
            qlocw=np.ascontiguousarray(qlocw),
            biasP=np.ascontiguousarray(biasT),
            rlocP=np.ascontiguousarray(rlocT),
        ))
    return T_BLK, NT, NCH, cores


def _prep_weights(inp):
    scale = 1.0 / math.sqrt(D)

    def mat(w, kchunks):
        w = np.asarray(w, np.float32)
        k, n = w.shape
        assert k == kchunks * 128
        return np.ascontiguousarray(
            w.reshape(kchunks, 128, n).transpose(1, 0, 2)).astype(bfloat16)

    def rowv(b):
        return np.asarray(b, np.float32)[None, :].astype(bfloat16)

    return dict(
        wq=mat(np.asarray(inp["Wq"], np.float32) * scale, 4),
        wk=mat(inp["Wk"], 4),
        wv=mat(inp["Wv"], 4),
        wo=mat(inp["Wo"], 4),
        w1=mat(inp["W1"], 4),
        w2=mat(inp["W2"], 8),
        bq=rowv(np.asarray(inp["bq"], np.float32) * scale),
        bk=rowv(inp["bk"]), bv=rowv(inp["bv"]), bo=rowv(inp["bo"]),
        b1=rowv(inp["b1"]), b2=rowv(inp["b2"]),
        ln1g=np.asarray(inp["ln1_g"], np.float32),
        ln1b=np.asarray(inp["ln1_b"], np.float32),
        ln2g=np.asarray(inp["ln2_g"], np.float32),
        ln2b=np.asarray(inp["ln2_b"], np.float32),
    )


# --------------------------------------------------------------------------
# walrus workaround: this walrus build rejects Drain instructions carrying
# more than one sem wait ("Too many sync wait commands") -- split the extra
# waits onto NOPs inserted just before, on the same engine.
# --------------------------------------------------------------------------

def _split_multi_waits(nc):
    nid = [0]
    for fn in nc.m.functions:
        for blk in fn.blocks:
            insts = blk.instructions
            i = 0
            while i < len(insts):
                inst = insts[i]
                si = inst.sync_info
                if (isinstance(inst, mybir.InstDrain)
                        and si is not None and si.on_wait and len(si.on_wait) > 1):
                    waits = list(si.on_wait)
                    nops = []
                    for w in waits[:-1]:
                        nid[0] += 1
                        nops.append(mybir.InstNoOp(
                            name=f"I-waitfix-{nid[0]}",
                            engine=inst.engine, ins=[], outs=[],
                            sync_info=mybir.SyncInfo(on_wait=[w], on_update=[]),
                        ))
                    inst.sync_info = mybir.SyncInfo(
                        on_wait=[waits[-1]], on_update=list(si.on_update))
                    insts[i:i] = nops
                    i += len(nops)
                i += 1


# --------------------------------------------------------------------------
# device program
# --------------------------------------------------------------------------

def _bc(ap, n):
    """append a broadcast (step-0) innermost dim of size n to an AP"""
    return bass.AP(tensor=ap.tensor, offset=ap.offset, ap=[*ap.ap, [0, n]])


def _phd(ap):
    return ap.rearrange("p (h d) -> p h d", h=H)


def _build_program(L, T_BLK, NT, NCH):
    LSH = L // NCORES
    NBLK = LSH // 128
    nc = bacc.Bacc(num_devices=NCORES)

    x_c = nc.declare_dram_parameter("x_c", [LSH, C], F32, isOutput=False)
    wq = nc.declare_dram_parameter("wq", [128, 4, C], BF16, isOutput=False)
    wk = nc.declare_dram_parameter("wk", [128, 4, C], BF16, isOutput=False)
    wv = nc.declare_dram_parameter("wv", [128, 4, C], BF16, isOutput=False)
    wo = nc.declare_dram_parameter("wo", [128, 4, C], BF16, isOutput=False)
    w1 = nc.declare_dram_parameter("w1", [128, 4, HID], BF16, isOutput=False)
    w2 = nc.declare_dram_parameter("w2", [128, 8, C], BF16, isOutput=False)
    bqp = nc.declare_dram_parameter("bq", [1, C], BF16, isOutput=False)
    bkp = nc.declare_dram_parameter("bk", [1, C], BF16, isOutput=False)
    bvp = nc.declare_dram_parameter("bv", [1, C], BF16, isOutput=False)
    bop = nc.declare_dram_parameter("bo", [1, C], BF16, isOutput=False)
    b1p = nc.declare_dram_parameter("b1", [1, HID], BF16, isOutput=False)
    b2p = nc.declare_dram_parameter("b2", [1, C], BF16, isOutput=False)
    ln1g = nc.declare_dram_parameter("ln1g", [C], F32, isOutput=False)
    ln1b = nc.declare_dram_parameter("ln1b", [C], F32, isOutput=False)
    ln2g = nc.declare_dram_parameter("ln2g", [C], F32, isOutput=False)
    ln2b = nc.declare_dram_parameter("ln2b", [C], F32, isOutput=False)
    colw = nc.declare_dram_parameter("colw", [NCH * 128, CHUNK_T * 8], I16, isOutput=False)
    qlocw = nc.declare_dram_parameter("qlocw", [NCH * 128, CHUNK_T * 8], I16, isOutput=False)
    biasP = nc.declare_dram_parameter("biasP", [NT, 128, H], F32, isOutput=False)
    rlocP = nc.declare_dram_parameter("rlocP", [NT, 128], F32, isOutput=False)
    y_out = nc.declare_dram_parameter("y", [LSH, C], F32, isOutput=True)

    with ExitStack() as ctx:
        tc = ctx.enter_context(tile.TileContext(nc))

        dram = ctx.enter_context(tc.tile_pool(name="dram", bufs=1, space="DRAM"))
        q_tab = dram.tile([LSH, C], BF16)
        kv_sh = dram.tile([LSH, 2 * C], BF16)
        kv_full = dram.tile([NCORES * LSH, 2 * C], BF16)
        x1_d = dram.tile([LSH, C], F32)

        # ---------------- constants + weights ----------------
        consts = ctx.enter_context(tc.tile_pool(name="consts", bufs=1))
        ident = consts.tile([128, 128], BF16, tag="ident")
        make_identity(nc, ident[:])
        iota_row = consts.tile([128, 128], BF16, tag="iota")
        nc.gpsimd.iota(iota_row[:], pattern=[[1, 128]], base=0,
                       channel_multiplier=0,
                       allow_small_or_imprecise_dtypes=True)
        ones_k1 = consts.tile([1, 128], BF16, tag="ones")
        nc.vector.memset(ones_k1[:], 1.0)
        eps_t = consts.tile([128, 1], F32, tag="eps")
        nc.vector.memset(eps_t[:], EPS)

        def bcast_load(param, tag):
            t = consts.tile([128, C], F32, tag=tag)
            ap = param[:]
            src = bass.AP(tensor=ap.tensor, offset=ap.offset,
                          ap=[[0, 128], [1, C]])
            nc.sync.dma_start(out=t[:], in_=src)
            return t

        g1_bc, b1_bc = bcast_load(ln1g, "g1"), bcast_load(ln1b, "b1")
        g2_bc, b2_bc = bcast_load(ln2g, "g2"), bcast_load(ln2b, "b2")

        wts = ctx.enter_context(tc.tile_pool(name="wts", bufs=1))

        def wload(p, shape, tag):
            t = wts.tile(shape, BF16, tag=tag)
            nc.sync.dma_start(out=t[:], in_=p[:])
            return t

        wq_sb = wload(wq, [128, 4, C], "wq"); wk_sb = wload(wk, [128, 4, C], "wk")
        wv_sb = wload(wv, [128, 4, C], "wv"); wo_sb = wload(wo, [128, 4, C], "wo")
        w1_sb = wload(w1, [128, 4, HID], "w1"); w2_sb = wload(w2, [128, 8, C], "w2")
        bq_sb = wload(bqp, [1, C], "bq"); bk_sb = wload(bkp, [1, C], "bk")
        bv_sb = wload(bvp, [1, C], "bv"); bo_sb = wload(bop, [1, C], "bo")
        b1_sb = wload(b1p, [1, HID], "bb1"); b2_sb = wload(b2p, [1, C], "bb2")

        # ---------------- LN helper ----------------
        def layernorm(pool, lnpool, xb, g_bc, bb_bc):
            """returns bf16 [128, C] normalized tile"""
            stats = lnpool.tile([128, 6], F32, tag="stats")
            nc.vector.bn_stats(stats[:], xb[:])
            mv = lnpool.tile([128, 2], F32, tag="mv")
            nc.vector.bn_aggr(mv[:], stats[:])
            xc = pool.tile([128, C], F32, tag="ln_xc")
            nc.vector.tensor_scalar(xc[:], xb[:], mv[:, 0:1], None, op0=ALU.subtract)
            sd = lnpool.tile([128, 1], F32, tag="sd")
            nc.scalar.activation(sd[:], mv[:, 1:2], AF.Sqrt, bias=eps_t[:])
            rstd = lnpool.tile([128, 1], F32, tag="rstd")
            nc.vector.reciprocal(rstd[:], sd[:])
            z0 = pool.tile([128, C], F32, tag="ln_z0")
            nc.vector.tensor_scalar(z0[:], xc[:], rstd[:], None, op0=ALU.mult)
            z1 = pool.tile([128, C], F32, tag="ln_z1")
            nc.vector.tensor_mul(z1[:], z0[:], g_bc[:])
            zb = pool.tile([128, C], BF16, tag="ln_out")
            nc.vector.tensor_add(zb[:], z1[:], bb_bc[:])
            return zb

        # ---------------- phase B+C: LN1, zT, QKV ----------------
        with ExitStack() as pctx:
            zT_pool = pctx.enter_context(tc.tile_pool(name="zT", bufs=1))
            zT = zT_pool.tile([128, 4, LSH], BF16)
            xp = pctx.enter_context(tc.tile_pool(name="xp", bufs=3))
            lnp = pctx.enter_context(tc.tile_pool(name="lnp", bufs=4))
            trp = pctx.enter_context(tc.tile_pool(name="trp", bufs=2, space="PSUM"))
            qkvp = pctx.enter_context(tc.tile_pool(name="qkvp", bufs=2, space="PSUM"))
            obp = pctx.enter_context(tc.tile_pool(name="obp", bufs=3))

            for ib in range(NBLK):
                sl = slice(ib * 128, (ib + 1) * 128)
                xb = xp.tile([128, C], F32, tag="xin")
                nc.sync.dma_start(out=xb[:], in_=x_c[sl, :])
                zb = layernorm(xp, lnp, xb, g1_bc, b1_bc)
                for g in range(4):
                    pt = trp.tile([128, 128], BF16)
                    nc.tensor.transpose(pt[:], zb[:, g * 128:(g + 1) * 128], ident[:])
                    nc.scalar.copy(zT[:, g, sl], pt[:])
                for w_sb, bias_sb, dst in (
                    (wq_sb, bq_sb, None),
                    (wk_sb, bk_sb, 0),
                    (wv_sb, bv_sb, 1),
                ):
                    ps = qkvp.tile([128, C], F32)
                    for g in range(4):
                        nc.tensor.matmul(ps[:], lhsT=zT[:, g, sl], rhs=w_sb[:, g, :],
                                         start=(g == 0), stop=False)
                    nc.tensor.matmul(ps[:], lhsT=ones_k1[:], rhs=bias_sb[:],
                                     start=False, stop=True)
                    ob = obp.tile([128, C], BF16)
                    nc.scalar.copy(ob[:], ps[:])
                    if dst is None:
                        nc.sync.dma_start(out=q_tab[sl, :], in_=ob[:])
                    else:
                        nc.sync.dma_start(out=kv_sh[sl, dst * C:(dst + 1) * C], in_=ob[:])

        # ---------------- phase D: allgather K/V ----------------
        nc.gpsimd.collective_compute(
            "AllGather", ALU.bypass,
            replica_groups=[list(range(NCORES))],
            ins=[kv_sh[:]], outs=[kv_full[:]],
        )

        # ---------------- phase E: edges ----------------
        with ExitStack() as pctx:
            kvp = pctx.enter_context(tc.tile_pool(name="kvp", bufs=2))
            qgp = pctx.enter_context(tc.tile_pool(name="qgp", bufs=2))
            idxp = pctx.enter_context(tc.tile_pool(name="idxp", bufs=3))
            bp = pctx.enter_context(tc.tile_pool(name="bp", bufs=2))
            rlp = pctx.enter_context(tc.tile_pool(name="rlp", bufs=2))
            work = pctx.enter_context(tc.tile_pool(name="work", bufs=4))
            pop_ = pctx.enter_context(tc.tile_pool(name="pout", bufs=2, space="PSUM"))
            psp = pctx.enter_context(tc.tile_pool(name="pssum", bufs=1, space="PSUM"))
            trp2 = pctx.enter_context(tc.tile_pool(name="trp2", bufs=2, space="PSUM"))
            opp = pctx.enter_context(tc.tile_pool(name="opsum", bufs=1, space="PSUM"))
            finp = pctx.enter_context(tc.tile_pool(name="finp", bufs=2))

            kvb = qgb = bia = rlc = None
            pout = pssum = None
            for t in range(NT):
                ch, slot = divmod(t, CHUNK_T)
                if slot == 0:
                    tiles_c = min(CHUNK_T, NT - ch * CHUNK_T)
                    n_idx = tiles_c * 128
                    cidx = idxp.tile([128, CHUNK_T * 8], I16, tag="cidx")
                    nc.sync.dma_start(out=cidx[:], in_=colw[ch * 128:(ch + 1) * 128, :])
                    qidx = idxp.tile([128, CHUNK_T * 8], I16, tag="qidx")
                    nc.sync.dma_start(out=qidx[:], in_=qlocw[ch * 128:(ch + 1) * 128, :])
                    kvb = kvp.tile([128, CHUNK_T, 2 * C], BF16)
                    nc.gpsimd.dma_gather(
                        out_ap=kvb[:, :tiles_c, :], in_ap=kv_full[:],
                        idxs_ap=cidx[:, :n_idx // 16],
                        num_idxs=n_idx, num_idxs_reg=n_idx, elem_size=2 * C,
                        single_packet=False)
                    qgb = qgp.tile([128, CHUNK_T, C], BF16)
                    nc.gpsimd.dma_gather(
                        out_ap=qgb[:, :tiles_c, :], in_ap=q_tab[:],
                        idxs_ap=qidx[:, :n_idx // 16],
                        num_idxs=n_idx, num_idxs_reg=n_idx, elem_size=C,
                        single_packet=False)
                    bia = bp.tile([128, CHUNK_T, H], F32)
                    nc.sync.dma_start(
                        out=bia[:, :tiles_c, :],
                        in_=biasP[ch * CHUNK_T:ch * CHUNK_T + tiles_c, :, :]
                        .rearrange("t p h -> p t h"))
                    rlc = rlp.tile([128, CHUNK_T], F32)
                    nc.sync.dma_start(
                        out=rlc[:, :tiles_c],
                        in_=rlocP[ch * CHUNK_T:ch * CHUNK_T + tiles_c, :]
                        .rearrange("t p -> p t"))

                rb, tb = divmod(t, T_BLK)
                if tb == 0:
                    pout = pop_.tile([128, C], F32)
                    pssum = psp.tile([128, H], F32)

                kg = kvb[:, slot, 0:C]
                vg = kvb[:, slot, C:2 * C]
                qg = qgb[:, slot, :]
                prod = work.tile([128, C], BF16, tag="prod")
                nc.vector.tensor_mul(prod[:], kg, qg)
                sc = work.tile([128, H], F32, tag="sc")
                nc.vector.tensor_reduce(sc[:], _phd(prod[:]), axis=AX.X, op=ALU.add)
                sc2 = work.tile([128, H], F32, tag="sc2")
                nc.vector.tensor_add(sc2[:], sc[:], bia[:, slot, :])
                p_t = work.tile([128, H], BF16, tag="p")
                nc.scalar.activation(p_t[:], sc2[:], AF.Exp)
                oh = work.tile([128, 128], BF16, tag="oh")
                nc.vector.tensor_scalar(oh[:], iota_row[:], rlc[:, slot:slot + 1],
                                        None, op0=ALU.is_equal)
                wt = work.tile([128, C], BF16, tag="wt")
                nc.vector.tensor_tensor(_phd(wt[:]), _phd(vg), _bc(p_t[:], D),
                                        op=ALU.mult)
                nc.tensor.matmul(pout[:], lhsT=oh[:], rhs=wt[:],
                                 start=(tb == 0), stop=(tb == T_BLK - 1))
                nc.tensor.matmul(pssum[:], lhsT=oh[:], rhs=p_t[:],
                                 start=(tb == 0), stop=(tb == T_BLK - 1))

                if tb == T_BLK - 1:
                    sl = slice(rb * 128, (rb + 1) * 128)
                    sm = finp.tile([128, H], F32, tag="sm")
                    nc.vector.tensor_scalar(sm[:], pssum[:], 1e-30, None, op0=ALU.max)
                    rec = finp.tile([128, H], F32, tag="rec")
                    nc.vector.reciprocal(rec[:], sm[:])
                    att = finp.tile([128, C], BF16, tag="att")
                    nc.vector.tensor_tensor(_phd(att[:]), _phd(pout[:]),
                                            _bc(rec[:], D), op=ALU.mult)
                    attT = finp.tile([128, 4, 128], BF16, tag="attT")
                    for g in range(4):
                        pt = trp2.tile([128, 128], BF16)
                        nc.tensor.transpose(pt[:], att[:, g * 128:(g + 1) * 128], ident[:])
                        nc.scalar.copy(attT[:, g, :], pt[:])
                    po = opp.tile([128, C], F32)
                    for g in range(4):
                        nc.tensor.matmul(po[:], lhsT=attT[:, g, :], rhs=wo_sb[:, g, :],
                                         start=(g == 0), stop=False)
                    nc.tensor.matmul(po[:], lhsT=ones_k1[:], rhs=bo_sb[:],
                                     start=False, stop=True)
                    xb2 = finp.tile([128, C], F32, tag="xb2")
                    nc.sync.dma_start(out=xb2[:], in_=x_c[sl, :])
                    x1t = finp.tile([128, C], F32, tag="x1t")
                    nc.vector.tensor_add(x1t[:], po[:], xb2[:])
                    nc.sync.dma_start(out=x1_d[sl, :], in_=x1t[:])

        # ---------------- phase F: LN2 + MLP ----------------
        with ExitStack() as pctx:
            xp = pctx.enter_context(tc.tile_pool(name="xp2", bufs=3))
            lnp = pctx.enter_context(tc.tile_pool(name="lnp2", bufs=4))
            trp3 = pctx.enter_context(tc.tile_pool(name="trp3", bufs=2, space="PSUM"))
            hp = pctx.enter_context(tc.tile_pool(name="hpsum", bufs=1, space="PSUM"))
            yp = pctx.enter_context(tc.tile_pool(name="ypsum", bufs=1, space="PSUM"))
            sbp = pctx.enter_context(tc.tile_pool(name="sbp", bufs=3))

            for ib in range(NBLK):
                sl = slice(ib * 128, (ib + 1) * 128)
                x1t = xp.tile([128, C], F32, tag="x1in")
                nc.sync.dma_start(out=x1t[:], in_=x1_d[sl, :])
                z2 = layernorm(xp, lnp, x1t, g2_bc, b2_bc)
                z2T = sbp.tile([128, 4, 128], BF16, tag="z2T")
                for g in range(4):
                    pt = trp3.tile([128, 128], BF16)
                    nc.tensor.transpose(pt[:], z2[:, g * 128:(g + 1) * 128], ident[:])
                    nc.scalar.copy(z2T[:, g, :], pt[:])
                ph = hp.tile([128, 8, 128], F32)
                for chc in range(8):
                    csl = slice(chc * 128, (chc + 1) * 128)
                    for g in range(4):
                        nc.tensor.matmul(ph[:, chc, :], lhsT=w1_sb[:, g, csl],
                                         rhs=z2T[:, g, :], start=(g == 0), stop=False)
                    nc.tensor.matmul(ph[:, chc, :], lhsT=b1_sb[:, csl],
                                     rhs=ones_k1[:], start=False, stop=True)
                hs = sbp.tile([128, 8, 128], BF16, tag="hs")
                nc.scalar.activation(hs[:], ph[:], AF.Silu)
                py = yp.tile([128, C], F32)
                for chc in range(8):
                    nc.tensor.matmul(py[:], lhsT=hs[:, chc, :], rhs=w2_sb[:, chc, :],
                                     start=(chc == 0), stop=False)
                nc.tensor.matmul(py[:], lhsT=ones_k1[:], rhs=b2_sb[:],
                                 start=False, stop=True)
                yt = sbp.tile([128, C], F32, tag="yt")
                nc.vector.tensor_add(yt[:], py[:], x1t[:])
                nc.sync.dma_start(out=y_out[sl, :], in_=yt[:])

    nc.finalize()
    _split_multi_waits(nc)
    return nc


# --------------------------------------------------------------------------
# entry point
# --------------------------------------------------------------------------

def kernel(**inputs) -> np.ndarray:
    x = np.asarray(inputs["x"], np.float32)
    row = np.asarray(inputs["row_index"]).astype(np.int64)
    col = np.asarray(inputs["col_index"]).astype(np.int64)
    att_bias = np.asarray(inputs["att_bias"], np.float32)
    L = x.shape[0]
    LSH = L // NCORES

    T_BLK, NT, NCH, cores = _preprocess_edges(L, row, col, att_bias)
    w = _prep_weights(inputs)

    key = (L, T_BLK, NT, NCH)
    if key not in _prog_cache:
        _prog_cache[key] = _build_program(L, T_BLK, NT, NCH)
    nc = _prog_cache[key]

    in_maps = []
    for c in range(NCORES):
        m = dict(w)
        m["x_c"] = np.ascontiguousarray(x[c * LSH:(c + 1) * LSH])
        m.update(cores[c])
        in_maps.append(m)

    global LAST_EXEC_NS, LAST_RESULTS
    res = run_bass_kernel_spmd(nc, in_maps, list(range(NCORES)), trace=TRACE)
    LAST_RESULTS = res
    LAST_EXEC_NS = res.exec_time_ns
    return np.concatenate([res.results[c]["y"] for c in range(NCORES)], axis=0)



# revision 25
# speedup vs baseline: 1.0080x; 1.0080x over previous
"""Trainium2 Bass kernel for a sparse-attention EncoderLayer.

Sharding: rows (L) are split into 8 contiguous shards of L/8; each edge is
owned by the core that owns its destination row (row_index is sorted, so each
core's edges are a contiguous range).  Each core computes Q/K/V for its row
shard, the K/V shards are AllGathered (bf16) so every core holds the full
K/V table in HBM, and per-edge K/V rows are fetched with dma_gather.  The
segment softmax is computed without the max-subtraction (scores here are
bounded by ~|q||k|/8 + |bias| < 10, so exp() cannot overflow in f32 and
alpha = exp(s - m)/sum exp(s - m) == exp(s)/sum exp(s)).  The alpha-weighted
scatter and the per-row softmax sums are evaluated as one-hot PE matmuls over
128-edge tiles, accumulated in PSUM per 128-row block.
"""

import math
import numpy as np
from contextlib import ExitStack

from ml_dtypes import bfloat16

import concourse.bass as bass
import concourse.mybir as mybir
import concourse.tile as tile
from concourse import bacc
from concourse.bass_utils import run_bass_kernel_spmd
from concourse.masks import make_identity

NCORES = 8
C, H, D, HID = 512, 8, 64, 1024
EPS = 1e-5
CHUNK_T = 16  # edge tiles (of 128 edges) per dma_gather chunk
F32 = mybir.dt.float32
BF16 = mybir.dt.bfloat16
FP8 = mybir.dt.float8e4   # TRN E4M3 (max normal 240)
I16 = mybir.dt.int16
VSCALE = 16.0             # V stored as fp8 * VSCALE; folded out via Wo
ROWB = 2 * 512 + 512      # kv_full row bytes: K bf16 (1024B) | V fp8 (512B)
AF = mybir.ActivationFunctionType
ALU = mybir.AluOpType
AX = mybir.AxisListType

_prog_cache = {}
TRACE = False          # set True (with the ntff hook registered) to profile
LAST_EXEC_NS = None    # exec time of the last run when TRACE was on
LAST_RESULTS = None    # full BassKernelResults of the last run


# --------------------------------------------------------------------------
# host-side preprocessing
# --------------------------------------------------------------------------

def _wrap_idx(idx):
    """[n] int -> [128, n//16] int16, wrapped (idx i at partition i%16,
    column i//16) and replicated across the 8 Q7 cores."""
    n = idx.shape[0]
    w = np.ascontiguousarray(idx.reshape(n // 16, 16).T).astype(np.int16)
    return np.tile(w, (8, 1))


def _preprocess_edges(L, row, col, att_bias):
    LSH = L // NCORES
    NBLK = LSH // 128
    bounds = np.searchsorted(row, np.arange(NCORES + 1) * LSH)

    per_core = []
    t_blk = 1
    for c in range(NCORES):
        e0, e1 = int(bounds[c]), int(bounds[c + 1])
        r = row[e0:e1] - c * LSH
        blk = r >> 7
        cnt = np.bincount(blk, minlength=NBLK)
        t_blk = max(t_blk, int(np.max((cnt + 127) // 128)) if len(cnt) else 1)
        per_core.append((e0, e1, r, blk, cnt))

    T_BLK = t_blk
    NT = NBLK * T_BLK
    NCH = (NT + CHUNK_T - 1) // CHUNK_T
    NTP = NCH * CHUNK_T  # tiles padded to whole chunks (extra tiles unused)

    cores = []
    for c in range(NCORES):
        e0, e1, r, blk, cnt = per_core[c]
        ne = e1 - e0
        starts = np.zeros(NBLK, dtype=np.int64)
        np.cumsum(cnt[:-1], out=starts[1:])
        idx_in_blk = np.arange(ne, dtype=np.int64) - starts[blk]
        dst = blk * (T_BLK * 128) + idx_in_blk

        npad = NTP * 128
        colP = np.zeros(npad, dtype=np.int64)
        biasP = np.full((npad, H), -30000.0, dtype=np.float32)
        colP[dst] = col[e0:e1]
        biasP[dst] = att_bias[e0:e1]

        # one-hot (edge -> local row in 128-block) and its transpose, bf16,
        # pre-swizzled to [NCH*128, CHUNK_T*128] so chunk loads are
        # contiguous 4KB-per-partition DMAs
        oh = np.zeros((npad, 128), np.uint16)
        oh[dst, r & 127] = 0x3F80  # bf16 1.0
        oh4 = oh.reshape(NCH, CHUNK_T, 128, 128)
        ohS = oh4.transpose(0, 2, 1, 3).reshape(NCH * 128, CHUNK_T * 128)
        ohT4 = oh4.transpose(0, 3, 1, 2).reshape(NCH * 128, CHUNK_T * 128)

        colw = _wrap_idx(colP).reshape(128, NCH, CHUNK_T * 8).transpose(1, 0, 2)
        colw = colw.reshape(NCH * 128, CHUNK_T * 8)
        biasT = (biasP.reshape(NCH, CHUNK_T, 128, H).transpose(0, 2, 1, 3)
                 .reshape(NCH * 128, CHUNK_T * H).astype(bfloat16))
        cores.append(dict(
            colw=np.ascontiguousarray(colw),
            biasP=np.ascontiguousarray(biasT),
            ohP=np.ascontiguousarray(ohS).view(bfloat16),
            ohTP=np.ascontiguousarray(ohT4).view(bfloat16),
        ))
    return T_BLK, NT, NCH, cores


def _prep_weights(inp):
    scale = 1.0 / math.sqrt(D)

    def mat(w, kchunks):
        w = np.asarray(w, np.float32)
        k, n = w.shape
        assert k == kchunks * 128
        return np.ascontiguousarray(
            w.reshape(kchunks, 128, n).transpose(1, 0, 2)).astype(bfloat16)

    def rowv(b):
        return np.asarray(b, np.float32)[None, :].astype(bfloat16)

    return dict(
        wq=mat(np.asarray(inp["Wq"], np.float32) * scale, 4),
        wk=mat(inp["Wk"], 4),
        wv=mat(np.asarray(inp["Wv"], np.float32) * VSCALE, 4),
        wo=mat(np.asarray(inp["Wo"], np.float32) / VSCALE, 4),
        w1=mat(inp["W1"], 4),
        w2=mat(inp["W2"], 8),
        bq=rowv(np.asarray(inp["bq"], np.float32) * scale),
        bk=rowv(inp["bk"]),
        bv=rowv(np.asarray(inp["bv"], np.float32) * VSCALE),
        bo=rowv(inp["bo"]),
        b1=rowv(inp["b1"]), b2=rowv(inp["b2"]),
        ln1g=np.asarray(inp["ln1_g"], np.float32),
        ln1b=np.asarray(inp["ln1_b"], np.float32),
        ln2g=np.asarray(inp["ln2_g"], np.float32),
        ln2b=np.asarray(inp["ln2_b"], np.float32),
    )


# --------------------------------------------------------------------------
# walrus workaround: this walrus build rejects Drain instructions carrying
# more than one sem wait ("Too many sync wait commands") -- split the extra
# waits onto NOPs inserted just before, on the same engine.
# --------------------------------------------------------------------------

def _split_multi_waits(nc):
    nid = [0]
    for fn in nc.m.functions:
        for blk in fn.blocks:
            insts = blk.instructions
            i = 0
            while i < len(insts):
                inst = insts[i]
                si = inst.sync_info
                if (isinstance(inst, mybir.InstDrain)
                        and si is not None and si.on_wait and len(si.on_wait) > 1):
                    waits = list(si.on_wait)
                    nops = []
                    for w in waits[:-1]:
                        nid[0] += 1
                        nops.append(mybir.InstNoOp(
                            name=f"I-waitfix-{nid[0]}",
                            engine=inst.engine, ins=[], outs=[],
                            sync_info=mybir.SyncInfo(on_wait=[w], on_update=[]),
                        ))
                    inst.sync_info = mybir.SyncInfo(
                        on_wait=[waits[-1]], on_update=list(si.on_update))
                    insts[i:i] = nops
                    i += len(nops)
                i += 1


# --------------------------------------------------------------------------
# device program
# --------------------------------------------------------------------------

def _bc(ap, n):
    """append a broadcast (step-0) innermost dim of size n to an AP"""
    return bass.AP(tensor=ap.tensor, offset=ap.offset, ap=[*ap.ap, [0, n]])


def _phd(ap):
    return ap.rearrange("p (h d) -> p h d", h=H)


def _build_program(L, T_BLK, NT, NCH):
    LSH = L // NCORES
    NBLK = LSH // 128
    nc = bacc.Bacc(num_devices=NCORES)

    x_c = nc.declare_dram_parameter("x_c", [LSH, C], F32, isOutput=False)
    wq = nc.declare_dram_parameter("wq", [128, 4, C], BF16, isOutput=False)
    wk = nc.declare_dram_parameter("wk", [128, 4, C], BF16, isOutput=False)
    wv = nc.declare_dram_parameter("wv", [128, 4, C], BF16, isOutput=False)
    wo = nc.declare_dram_parameter("wo", [128, 4, C], BF16, isOutput=False)
    w1 = nc.declare_dram_parameter("w1", [128, 4, HID], BF16, isOutput=False)
    w2 = nc.declare_dram_parameter("w2", [128, 8, C], BF16, isOutput=False)
    bqp = nc.declare_dram_parameter("bq", [1, C], BF16, isOutput=False)
    bkp = nc.declare_dram_parameter("bk", [1, C], BF16, isOutput=False)
    bvp = nc.declare_dram_parameter("bv", [1, C], BF16, isOutput=False)
    bop = nc.declare_dram_parameter("bo", [1, C], BF16, isOutput=False)
    b1p = nc.declare_dram_parameter("b1", [1, HID], BF16, isOutput=False)
    b2p = nc.declare_dram_parameter("b2", [1, C], BF16, isOutput=False)
    ln1g = nc.declare_dram_parameter("ln1g", [C], F32, isOutput=False)
    ln1b = nc.declare_dram_parameter("ln1b", [C], F32, isOutput=False)
    ln2g = nc.declare_dram_parameter("ln2g", [C], F32, isOutput=False)
    ln2b = nc.declare_dram_parameter("ln2b", [C], F32, isOutput=False)
    colw = nc.declare_dram_parameter("colw", [NCH * 128, CHUNK_T * 8], I16, isOutput=False)
    biasP = nc.declare_dram_parameter("biasP", [NCH * 128, CHUNK_T * H], BF16, isOutput=False)
    ohP = nc.declare_dram_parameter("ohP", [NCH * 128, CHUNK_T * 128], BF16, isOutput=False)
    ohTP = nc.declare_dram_parameter("ohTP", [NCH * 128, CHUNK_T * 128], BF16, isOutput=False)
    y_out = nc.declare_dram_parameter("y", [LSH, C], F32, isOutput=True)

    with ExitStack() as ctx:
        tc = ctx.enter_context(tile.TileContext(nc))

        dram = ctx.enter_context(tc.tile_pool(name="dram", bufs=1, space="DRAM"))
        kv_sh = dram.tile([LSH, ROWB], FP8)
        kv_full = dram.tile([NCORES * LSH, ROWB], FP8)
        x1_d = dram.tile([LSH, C], F32)

        # ---------------- constants + weights ----------------
        consts = ctx.enter_context(tc.tile_pool(name="consts", bufs=1))
        ident = consts.tile([128, 128], BF16, tag="ident")
        make_identity(nc, ident[:])
        ones_k1 = consts.tile([1, 128], BF16, tag="ones")
        nc.vector.memset(ones_k1[:], 1.0)
        eps_t = consts.tile([128, 1], F32, tag="eps")
        nc.vector.memset(eps_t[:], EPS)

        def bcast_load(param, tag):
            t = consts.tile([128, C], F32, tag=tag)
            ap = param[:]
            src = bass.AP(tensor=ap.tensor, offset=ap.offset,
                          ap=[[0, 128], [1, C]])
            nc.sync.dma_start(out=t[:], in_=src)
            return t

        g1_bc, b1_bc = bcast_load(ln1g, "g1"), bcast_load(ln1b, "b1")
        g2_bc, b2_bc = bcast_load(ln2g, "g2"), bcast_load(ln2b, "b2")

        wts = ctx.enter_context(tc.tile_pool(name="wts", bufs=1))

        def wload(p, shape, tag):
            t = wts.tile(shape, BF16, tag=tag)
            nc.sync.dma_start(out=t[:], in_=p[:])
            return t

        wq_sb = wload(wq, [128, 4, C], "wq"); wk_sb = wload(wk, [128, 4, C], "wk")
        wv_sb = wload(wv, [128, 4, C], "wv"); wo_sb = wload(wo, [128, 4, C], "wo")
        w1_sb = wload(w1, [128, 4, HID], "w1"); w2_sb = wload(w2, [128, 8, C], "w2")
        bq_sb = wload(bqp, [1, C], "bq"); bk_sb = wload(bkp, [1, C], "bk")
        bv_sb = wload(bvp, [1, C], "bv"); bo_sb = wload(bop, [1, C], "bo")
        b1_sb = wload(b1p, [1, HID], "bb1"); b2_sb = wload(b2p, [1, C], "bb2")

        # Q table lives in SBUF for the whole kernel (gathered via one-hot PE
        # matmuls per edge tile, never written to DRAM)
        qall_pool = ctx.enter_context(tc.tile_pool(name="qall", bufs=1))
        q_all = qall_pool.tile([128, NBLK, C], BF16)

        # ---------------- LN helper ----------------
        def layernorm(pool, lnpool, xb, g_bc, bb_bc):
            """returns bf16 [128, C] normalized tile"""
            stats = lnpool.tile([128, 6], F32, tag="stats")
            nc.vector.bn_stats(stats[:], xb[:])
            mv = lnpool.tile([128, 2], F32, tag="mv")
            nc.vector.bn_aggr(mv[:], stats[:])
            xc = pool.tile([128, C], F32, tag="ln_xc")
            nc.vector.tensor_scalar(xc[:], xb[:], mv[:, 0:1], None, op0=ALU.subtract)
            sd = lnpool.tile([128, 1], F32, tag="sd")
            nc.scalar.activation(sd[:], mv[:, 1:2], AF.Sqrt, bias=eps_t[:])
            rstd = lnpool.tile([128, 1], F32, tag="rstd")
            nc.vector.reciprocal(rstd[:], sd[:])
            z0 = pool.tile([128, C], F32, tag="ln_z0")
            nc.vector.tensor_scalar(z0[:], xc[:], rstd[:], None, op0=ALU.mult)
            z1 = pool.tile([128, C], F32, tag="ln_z1")
            nc.vector.tensor_mul(z1[:], z0[:], g_bc[:])
            zb = pool.tile([128, C], BF16, tag="ln_out")
            nc.vector.tensor_add(zb[:], z1[:], bb_bc[:])
            return zb

        # ---------------- phase B+C: LN1, zT, QKV ----------------
        with ExitStack() as pctx:
            zT_pool = pctx.enter_context(tc.tile_pool(name="zT", bufs=1))
            zT = zT_pool.tile([128, 4, LSH], BF16)
            xp = pctx.enter_context(tc.tile_pool(name="xp", bufs=3))
            lnp = pctx.enter_context(tc.tile_pool(name="lnp", bufs=4))
            trp = pctx.enter_context(tc.tile_pool(name="trp", bufs=2, space="PSUM"))
            qkvp = pctx.enter_context(tc.tile_pool(name="qkvp", bufs=2, space="PSUM"))
            obp = pctx.enter_context(tc.tile_pool(name="obp", bufs=3))

            for ib in range(NBLK):
                sl = slice(ib * 128, (ib + 1) * 128)
                xb = xp.tile([128, C], F32, tag="xin")
                nc.sync.dma_start(out=xb[:], in_=x_c[sl, :])
                zb = layernorm(xp, lnp, xb, g1_bc, b1_bc)
                for g in range(4):
                    pt = trp.tile([128, 128], BF16)
                    nc.tensor.transpose(pt[:], zb[:, g * 128:(g + 1) * 128], ident[:])
                    nc.scalar.copy(zT[:, g, sl], pt[:])
                for w_sb, bias_sb, dst in (
                    (wq_sb, bq_sb, "q"),
                    (wk_sb, bk_sb, "k"),
                    (wv_sb, bv_sb, "v"),
                ):
                    ps = qkvp.tile([128, C], F32)
                    for g in range(4):
                        nc.tensor.matmul(ps[:], lhsT=zT[:, g, sl], rhs=w_sb[:, g, :],
                                         start=(g == 0), stop=False)
                    nc.tensor.matmul(ps[:], lhsT=ones_k1[:], rhs=bias_sb[:],
                                     start=False, stop=True)
                    if dst == "v":
                        ob = obp.tile([128, C], FP8, tag="ob_v")
                        nc.scalar.copy(ob[:], ps[:])
                        nc.sync.dma_start(out=kv_sh[sl, 2 * C:3 * C], in_=ob[:])
                    elif dst == "q":
                        nc.scalar.copy(q_all[:, ib, :], ps[:])
                    else:
                        ob = obp.tile([128, C], BF16, tag="ob_k")
                        nc.scalar.copy(ob[:], ps[:])
                        nc.sync.dma_start(
                            out=kv_sh[sl, 0:2 * C].bitcast(BF16), in_=ob[:])

        # ---------------- phase D: allgather K/V ----------------
        nc.gpsimd.collective_compute(
            "AllGather", ALU.bypass,
            replica_groups=[list(range(NCORES))],
            ins=[kv_sh[:]], outs=[kv_full[:]],
        )

        # ---------------- phase E: edges ----------------
        with ExitStack() as pctx:
            kvp = pctx.enter_context(tc.tile_pool(name="kvp", bufs=2))
            ohp = pctx.enter_context(tc.tile_pool(name="ohp", bufs=2))
            ohtp = pctx.enter_context(tc.tile_pool(name="ohtp", bufs=2))
            idxp = pctx.enter_context(tc.tile_pool(name="idxp", bufs=3))
            bp = pctx.enter_context(tc.tile_pool(name="bp", bufs=2))
            scp = pctx.enter_context(tc.tile_pool(name="scp", bufs=2))
            pcp = pctx.enter_context(tc.tile_pool(name="pcp", bufs=2))
            qsb = pctx.enter_context(tc.tile_pool(name="qsb", bufs=2))
            wkp = pctx.enter_context(tc.tile_pool(name="wkp", bufs=2))
            wtp = pctx.enter_context(tc.tile_pool(name="wtp", bufs=3))
            vup_p = pctx.enter_context(tc.tile_pool(name="vup", bufs=2))
            finp = pctx.enter_context(tc.tile_pool(name="finp", bufs=2))
            qgps = pctx.enter_context(tc.tile_pool(name="qgps", bufs=2, space="PSUM"))
            pop_ = pctx.enter_context(tc.tile_pool(name="pout", bufs=2, space="PSUM"))
            psp = pctx.enter_context(tc.tile_pool(name="pssum", bufs=1, space="PSUM"))
            opp = pctx.enter_context(tc.tile_pool(name="opsum", bufs=1, space="PSUM"))

            pout = pssum = None
            for ch in range(NCH):
                tiles_c = min(CHUNK_T, NT - ch * CHUNK_T)
                n_idx = tiles_c * 128
                cidx = idxp.tile([128, CHUNK_T * 8], I16, tag="cidx")
                nc.sync.dma_start(out=cidx[:], in_=colw[ch * 128:(ch + 1) * 128, :])
                kvb = kvp.tile([128, CHUNK_T, ROWB], FP8)
                nc.gpsimd.dma_gather(
                    out_ap=kvb[:, :tiles_c, :], in_ap=kv_full[:],
                    idxs_ap=cidx[:, :n_idx // 16],
                    num_idxs=n_idx, num_idxs_reg=n_idx, elem_size=ROWB,
                    single_packet=False)
                ohc = ohp.tile([128, CHUNK_T, 128], BF16)
                nc.sync.dma_start(
                    out=ohc[:],
                    in_=ohP[ch * 128:(ch + 1) * 128, :]
                    .rearrange("p (t r) -> p t r", t=CHUNK_T))
                ohtc = ohtp.tile([128, CHUNK_T, 128], BF16)
                nc.sync.dma_start(
                    out=ohtc[:],
                    in_=ohTP[ch * 128:(ch + 1) * 128, :]
                    .rearrange("p (t r) -> p t r", t=CHUNK_T))
                bia = bp.tile([128, CHUNK_T, H], BF16)
                nc.sync.dma_start(
                    out=bia[:],
                    in_=biasP[ch * 128:(ch + 1) * 128, :]
                    .rearrange("p (t h) -> p t h", t=CHUNK_T))

                # bf16 view of the K region of the gathered rows:
                # [128, CHUNK_T, ROWB/2 bf16]; K = cols 0:C of each row
                kvKb = kvb[:].bitcast(BF16)  # [128, CHUNK_T, ROWB//2 bf16]

                # pass 1: per-tile Q one-hot gather (PE) + pair-batched
                # mul / fold / reduce / bias-add scores (vector)
                scc = scp.tile([128, CHUNK_T, H], F32)
                for s2 in range((tiles_c + 1) // 2):
                    s0 = s2 * 2
                    ns = min(2, tiles_c - s0)
                    qg2 = qgps.tile([128, 2, C], F32)
                    for j in range(ns):
                        slot = s0 + j
                        rb = (ch * CHUNK_T + slot) // T_BLK
                        nc.tensor.matmul(qg2[:, j, :], lhsT=ohtc[:, slot, :],
                                         rhs=q_all[:, rb, :], start=True, stop=True)
                    qg_sb = qsb.tile([128, 2, C], BF16)
                    nc.scalar.copy(qg_sb[:, :ns, :], qg2[:, :ns, :])
                    prod = wkp.tile([128, 2, C], BF16, tag="prod")
                    nc.vector.tensor_tensor(
                        prod[:, :ns, :], kvKb[:, s0:s0 + ns, 0:C],
                        qg_sb[:, :ns, :], op=ALU.mult)
                    prodf = wkp.tile([128, 2, H, D // 2], BF16, tag="prodf")
                    pr4 = prod[:].rearrange("p t (h d) -> p t h d", h=H)
                    nc.vector.tensor_tensor(
                        prodf[:, :ns], pr4[:, :ns, :, 0:D // 2],
                        pr4[:, :ns, :, D // 2:D], op=ALU.add)
                    nc.vector.tensor_reduce(
                        scc[:, s0:s0 + ns, :], prodf[:, :ns],
                        axis=AX.X, op=ALU.add)
                    nc.vector.tensor_add(scc[:, s0:s0 + ns, :],
                                         scc[:, s0:s0 + ns, :],
                                         bia[:, s0:s0 + ns, :])
                pc = pcp.tile([128, CHUNK_T, H], BF16)
                nc.scalar.activation(pc[:, :tiles_c, :], scc[:, :tiles_c, :], AF.Exp)

                # pass 2: alpha-weighted scatter (one-hot PE matmuls).
                # wt pairs alternate between gpsimd (direct fp8+broadcast
                # read) and vector (scalar-engine upconvert+expand, 2x mode)
                for s2 in range((tiles_c + 1) // 2):
                    s0 = s2 * 2
                    ns = min(2, tiles_c - s0)
                    vg2 = kvb[:, s0:s0 + ns, 2 * C:3 * C]
                    wt2 = wtp.tile([128, 2, C], BF16, tag="wt")
                    if s2 % 2 == 0:
                        nc.gpsimd.tensor_tensor(
                            wt2[:, :ns, :].rearrange("p t (h d) -> p t h d", h=H),
                            vg2.rearrange("p t (h d) -> p t h d", h=H),
                            _bc(pc[:, s0:s0 + ns, :], D), op=ALU.mult)
                    else:
                        vup = vup_p.tile([128, 2, C], BF16, tag="vup")
                        nc.scalar.copy(vup[:, :ns, :], vg2)
                        pex = vup_p.tile([128, 2, C], BF16, tag="pex")
                        nc.scalar.copy(
                            pex[:, :ns, :].rearrange("p t (h d) -> p t h d", h=H),
                            _bc(pc[:, s0:s0 + ns, :], D))
                        nc.vector.tensor_tensor(wt2[:, :ns, :], vup[:, :ns, :],
                                                pex[:, :ns, :], op=ALU.mult)
                    for j in range(ns):
                        slot = s0 + j
                        t = ch * CHUNK_T + slot
                        rb, tb = divmod(t, T_BLK)
                        if tb == 0:
                            pout = pop_.tile([128, C], F32)
                            pssum = psp.tile([128, H], F32)
                        nc.tensor.matmul(pout[:], lhsT=ohc[:, slot, :],
                                         rhs=wt2[:, j, :],
                                         start=(tb == 0), stop=(tb == T_BLK - 1))
                        nc.tensor.matmul(pssum[:], lhsT=ohc[:, slot, :],
                                         rhs=pc[:, slot, :],
                                         start=(tb == 0), stop=(tb == T_BLK - 1))
                        if tb != T_BLK - 1:
                            continue
                        sl = slice(rb * 128, (rb + 1) * 128)
                        sm = finp.tile([128, H], F32, tag="sm")
                        nc.vector.tensor_scalar(sm[:], pssum[:], 1e-30, None, op0=ALU.max)
                        rec = finp.tile([128, H], F32, tag="rec")
                        nc.vector.reciprocal(rec[:], sm[:])
                        att = finp.tile([128, C], BF16, tag="att")
                        nc.vector.tensor_tensor(_phd(att[:]), _phd(pout[:]),
                                                _bc(rec[:], D), op=ALU.mult)
                        attT = finp.tile([128, 4, 128], BF16, tag="attT")
                        for g in range(4):
                            nc.sync.dma_start_transpose(
                                attT[:, g, :], att[:, g * 128:(g + 1) * 128])
                        po = opp.tile([128, C], F32)
                        for g in range(4):
                            nc.tensor.matmul(po[:], lhsT=attT[:, g, :], rhs=wo_sb[:, g, :],
                                             start=(g == 0), stop=False)
                        nc.tensor.matmul(po[:], lhsT=ones_k1[:], rhs=bo_sb[:],
                                         start=False, stop=True)
                        xb2 = finp.tile([128, C], F32, tag="xb2")
                        nc.sync.dma_start(out=xb2[:], in_=x_c[sl, :])
                        x1t = finp.tile([128, C], F32, tag="x1t")
                        nc.vector.tensor_add(x1t[:], po[:], xb2[:])
                        nc.sync.dma_start(out=x1_d[sl, :], in_=x1t[:])

        # ---------------- phase F: LN2 + MLP ----------------
        with ExitStack() as pctx:
            xp = pctx.enter_context(tc.tile_pool(name="xp2", bufs=3))
            lnp = pctx.enter_context(tc.tile_pool(name="lnp2", bufs=4))
            trp3 = pctx.enter_context(tc.tile_pool(name="trp3", bufs=2, space="PSUM"))
            hp = pctx.enter_context(tc.tile_pool(name="hpsum", bufs=1, space="PSUM"))
            yp = pctx.enter_context(tc.tile_pool(name="ypsum", bufs=1, space="PSUM"))
            sbp = pctx.enter_context(tc.tile_pool(name="sbp", bufs=3))

            for ib in range(NBLK):
                sl = slice(ib * 128, (ib + 1) * 128)
                x1t = xp.tile([128, C], F32, tag="x1in")
                nc.sync.dma_start(out=x1t[:], in_=x1_d[sl, :])
                z2 = layernorm(xp, lnp, x1t, g2_bc, b2_bc)
                z2T = sbp.tile([128, 4, 128], BF16, tag="z2T")
                for g in range(4):
                    pt = trp3.tile([128, 128], BF16)
                    nc.tensor.transpose(pt[:], z2[:, g * 128:(g + 1) * 128], ident[:])
                    nc.scalar.copy(z2T[:, g, :], pt[:])
                ph = hp.tile([128, 8, 128], F32)
                for chc in range(8):
                    csl = slice(chc * 128, (chc + 1) * 128)
                    for g in range(4):
                        nc.tensor.matmul(ph[:, chc, :], lhsT=w1_sb[:, g, csl],
                                         rhs=z2T[:, g, :], start=(g == 0), stop=False)
                    nc.tensor.matmul(ph[:, chc, :], lhsT=b1_sb[:, csl],
                                     rhs=ones_k1[:], start=False, stop=True)
                hs = sbp.tile([128, 8, 128], BF16, tag="hs")
                nc.scalar.activation(hs[:], ph[:], AF.Silu)
                py = yp.tile([128, C], F32)
                for chc in range(8):
                    nc.tensor.matmul(py[:], lhsT=hs[:, chc, :], rhs=w2_sb[:, chc, :],
                                     start=(chc == 0), stop=False)
                nc.tensor.matmul(py[:], lhsT=ones_k1[:], rhs=b2_sb[:],
                                 start=False, stop=True)
                yt = sbp.tile([128, C], F32, tag="yt")
                nc.vector.tensor_add(yt[:], py[:], x1t[:])
                nc.sync.dma_start(out=y_out[sl, :], in_=yt[:])

    nc.finalize()
    _split_multi_waits(nc)
    return nc


# --------------------------------------------------------------------------
# entry point
# --------------------------------------------------------------------------

def kernel(**inputs) -> np.ndarray:
    x = np.asarray(inputs["x"], np.float32)
    row = np.asarray(inputs["row_index"]).astype(np.int64)
    col = np.asarray(inputs["col_index"]).astype(np.int64)
    att_bias = np.asarray(inputs["att_bias"], np.float32)
    L = x.shape[0]
    LSH = L // NCORES

    T_BLK, NT, NCH, cores = _preprocess_edges(L, row, col, att_bias)
    w = _prep_weights(inputs)

    key = (L, T_BLK, NT, NCH)
    if key not in _prog_cache:
        _prog_cache[key] = _build_program(L, T_BLK, NT, NCH)
    nc = _prog_cache[key]

    in_maps = []
    for c in range(NCORES):
        m = dict(w)
        m["x_c"] = np.ascontiguousarray(x[c * LSH:(c + 1) * LSH])
        m.update(cores[c])
        in_maps.append(m)

    global LAST_EXEC_NS, LAST_RESULTS
    res = run_bass_kernel_spmd(nc, in_maps, list(range(NCORES)), trace=TRACE)
    LAST_RESULTS = res
    LAST_EXEC_NS = res.exec_time_ns
    return np.concatenate([res.results[c]["y"] for c in range(NCORES)], axis=0)



# revision 35
# speedup vs baseline: 1.4253x; 1.4141x over previous
"""Trainium2 Bass kernel for a sparse-attention EncoderLayer.

Sharding: rows (L) are split into 8 contiguous shards of L/8; each edge is
owned by the core that owns its destination row (row_index is sorted, so each
core's edges are a contiguous range).  Each core computes Q/K/V for its row
shard, the K/V shards are AllGathered (bf16) so every core holds the full
K/V table in HBM, and per-edge K/V rows are fetched with dma_gather.  The
segment softmax is computed without the max-subtraction (scores here are
bounded by ~|q||k|/8 + |bias| < 10, so exp() cannot overflow in f32 and
alpha = exp(s - m)/sum exp(s - m) == exp(s)/sum exp(s)).  The alpha-weighted
scatter and the per-row softmax sums are evaluated as one-hot PE matmuls over
128-edge tiles, accumulated in PSUM per 128-row block.
"""

import math
import numpy as np
from contextlib import ExitStack

from ml_dtypes import bfloat16

import concourse.bass as bass
import concourse.mybir as mybir
import concourse.tile as tile
from concourse import bacc
from concourse.bass_utils import run_bass_kernel_spmd
from concourse.masks import make_identity

NCORES = 8
C, H, D, HID = 512, 8, 64, 1024
EPS = 1e-5
CHUNK_T = 16  # edge tiles (of 128 edges) per dma_gather chunk
F32 = mybir.dt.float32
BF16 = mybir.dt.bfloat16
FP8 = mybir.dt.float8e4   # TRN E4M3 (max normal 240)
I16 = mybir.dt.int16
VSCALE = 16.0             # V stored as fp8 * VSCALE; folded out via Wo
ROWB = 2 * 512 + 512      # kv_full row bytes: K bf16 (1024B) | V fp8 (512B)
AF = mybir.ActivationFunctionType
ALU = mybir.AluOpType
AX = mybir.AxisListType

_prog_cache = {}
TRACE = False          # set True (with the ntff hook registered) to profile
LAST_EXEC_NS = None    # exec time of the last run when TRACE was on
LAST_RESULTS = None    # full BassKernelResults of the last run


# --------------------------------------------------------------------------
# host-side preprocessing
# --------------------------------------------------------------------------

def _wrap_idx(idx):
    """[n] int -> [128, n//16] int16, wrapped (idx i at partition i%16,
    column i//16) and replicated across the 8 Q7 cores."""
    n = idx.shape[0]
    w = np.ascontiguousarray(idx.reshape(n // 16, 16).T).astype(np.int16)
    return np.tile(w, (8, 1))


def _preprocess_edges(L, row, col, att_bias):
    """Variable tiles per 128-row block; each core's edges packed into its
    blocks' tiles back-to-back.  Gather table rows are remapped for the
    QSPLIT-way pipelined AllGather layout."""
    LSH = L // NCORES
    NBLK = LSH // 128
    QSPLIT = min(4, NBLK)
    ROWS_Q = LSH // QSPLIT
    bounds = np.searchsorted(row, np.arange(NCORES + 1) * LSH)

    per_core = []
    cnt_max = np.zeros(NBLK, dtype=np.int64)
    for c in range(NCORES):
        e0, e1 = int(bounds[c]), int(bounds[c + 1])
        r = row[e0:e1] - c * LSH
        blk = r >> 7
        cnt = np.bincount(blk, minlength=NBLK)
        np.maximum(cnt_max, cnt, out=cnt_max)
        per_core.append((e0, e1, r, blk, cnt))

    # one SPMD program for all cores: per-block tile count = max over cores
    nt_b = np.maximum((cnt_max + 127) // 128, 1)
    tstart = np.zeros(NBLK, dtype=np.int64)
    np.cumsum(nt_b[:-1], out=tstart[1:])
    NT = int(nt_b.sum())
    NCH = (NT + CHUNK_T - 1) // CHUNK_T
    NTP = NCH * CHUNK_T
    tmap = []
    for b in range(NBLK):
        for i in range(int(nt_b[b])):
            tmap.append((b, i == 0, i == int(nt_b[b]) - 1))
    tmap = tuple(tmap)

    cores = []
    for c in range(NCORES):
        e0, e1, r, blk, cnt = per_core[c]
        ne = e1 - e0
        starts = np.zeros(NBLK, dtype=np.int64)
        np.cumsum(cnt[:-1], out=starts[1:])
        idx_in_blk = np.arange(ne, dtype=np.int64) - starts[blk]
        dst = tstart[blk] * 128 + idx_in_blk

        npad = NTP * 128
        colP = np.zeros(npad, dtype=np.int64)
        biasP = np.full((npad, H), -30000.0, dtype=np.float32)
        # remap col -> row of the AG-split kv_full table
        cc = col[e0:e1]
        src_core = cc // LSH
        rloc = cc % LSH
        colP[dst] = ((rloc // ROWS_Q) * NCORES + src_core) * ROWS_Q + rloc % ROWS_Q
        biasP[dst] = att_bias[e0:e1]

        # one-hot (edge -> local row in 128-block) and its transpose, bf16,
        # pre-swizzled to [NCH*128, CHUNK_T*128] so chunk loads are
        # contiguous 4KB-per-partition DMAs
        oh = np.zeros((npad, 128), np.uint16)
        oh[dst, r & 127] = 0x3F80  # bf16 1.0
        oh4 = oh.reshape(NCH, CHUNK_T, 128, 128)
        ohS = oh4.transpose(0, 2, 1, 3).reshape(NCH * 128, CHUNK_T * 128)
        ohT4 = oh4.transpose(0, 3, 1, 2).reshape(NCH * 128, CHUNK_T * 128)

        colw = _wrap_idx(colP).reshape(128, NCH, CHUNK_T * 8).transpose(1, 0, 2)
        colw = colw.reshape(NCH * 128, CHUNK_T * 8)
        biasT = (biasP.reshape(NCH, CHUNK_T, 128, H).transpose(0, 2, 1, 3)
                 .reshape(NCH * 128, CHUNK_T * H).astype(bfloat16))
        cores.append(dict(
            colw=np.ascontiguousarray(colw),
            biasP=np.ascontiguousarray(biasT),
            ohP=np.ascontiguousarray(ohS).view(bfloat16),
            ohTP=np.ascontiguousarray(ohT4).view(bfloat16),
        ))
    return tmap, NT, NCH, cores


def _prep_weights(inp):
    scale = 1.0 / math.sqrt(D)

    def mat(w, kchunks):
        w = np.asarray(w, np.float32)
        k, n = w.shape
        assert k == kchunks * 128
        return np.ascontiguousarray(
            w.reshape(kchunks, 128, n).transpose(1, 0, 2)).astype(bfloat16)

    def rowv(b):
        return np.asarray(b, np.float32)[None, :].astype(bfloat16)

    return dict(
        wq=mat(np.asarray(inp["Wq"], np.float32) * scale, 4),
        wk=mat(inp["Wk"], 4),
        wv=mat(np.asarray(inp["Wv"], np.float32) * VSCALE, 4),
        wo=mat(np.asarray(inp["Wo"], np.float32) / VSCALE, 4),
        w1=mat(inp["W1"], 4),
        w2=mat(inp["W2"], 8),
        bq=rowv(np.asarray(inp["bq"], np.float32) * scale),
        bk=rowv(inp["bk"]),
        bv=rowv(np.asarray(inp["bv"], np.float32) * VSCALE),
        bo=rowv(inp["bo"]),
        b1=rowv(inp["b1"]), b2=rowv(inp["b2"]),
        ln1g=np.asarray(inp["ln1_g"], np.float32),
        ln1b=np.asarray(inp["ln1_b"], np.float32),
        ln2g=np.asarray(inp["ln2_g"], np.float32),
        ln2b=np.asarray(inp["ln2_b"], np.float32),
    )


# --------------------------------------------------------------------------
# walrus workaround: this walrus build rejects Drain instructions carrying
# more than one sem wait ("Too many sync wait commands") -- split the extra
# waits onto NOPs inserted just before, on the same engine.
# --------------------------------------------------------------------------

def _split_multi_waits(nc):
    nid = [0]
    for fn in nc.m.functions:
        for blk in fn.blocks:
            insts = blk.instructions
            i = 0
            while i < len(insts):
                inst = insts[i]
                si = inst.sync_info
                if (isinstance(inst, mybir.InstDrain)
                        and si is not None and si.on_wait and len(si.on_wait) > 1):
                    waits = list(si.on_wait)
                    nops = []
                    for w in waits[:-1]:
                        nid[0] += 1
                        nops.append(mybir.InstNoOp(
                            name=f"I-waitfix-{nid[0]}",
                            engine=inst.engine, ins=[], outs=[],
                            sync_info=mybir.SyncInfo(on_wait=[w], on_update=[]),
                        ))
                    inst.sync_info = mybir.SyncInfo(
                        on_wait=[waits[-1]], on_update=list(si.on_update))
                    insts[i:i] = nops
                    i += len(nops)
                i += 1


# --------------------------------------------------------------------------
# device program
# --------------------------------------------------------------------------

def _bc(ap, n):
    """append a broadcast (step-0) innermost dim of size n to an AP"""
    return bass.AP(tensor=ap.tensor, offset=ap.offset, ap=[*ap.ap, [0, n]])


def _phd(ap):
    return ap.rearrange("p (h d) -> p h d", h=H)


def _build_program(L, tmap, NT, NCH):
    LSH = L // NCORES
    NBLK = LSH // 128
    QSPLIT = min(4, NBLK)
    ROWS_Q = LSH // QSPLIT
    nc = bacc.Bacc(num_devices=NCORES)

    x_c = nc.declare_dram_parameter("x_c", [LSH, C], F32, isOutput=False)
    wq = nc.declare_dram_parameter("wq", [128, 4, C], BF16, isOutput=False)
    wk = nc.declare_dram_parameter("wk", [128, 4, C], BF16, isOutput=False)
    wv = nc.declare_dram_parameter("wv", [128, 4, C], BF16, isOutput=False)
    wo = nc.declare_dram_parameter("wo", [128, 4, C], BF16, isOutput=False)
    w1 = nc.declare_dram_parameter("w1", [128, 4, HID], BF16, isOutput=False)
    w2 = nc.declare_dram_parameter("w2", [128, 8, C], BF16, isOutput=False)
    bqp = nc.declare_dram_parameter("bq", [1, C], BF16, isOutput=False)
    bkp = nc.declare_dram_parameter("bk", [1, C], BF16, isOutput=False)
    bvp = nc.declare_dram_parameter("bv", [1, C], BF16, isOutput=False)
    bop = nc.declare_dram_parameter("bo", [1, C], BF16, isOutput=False)
    b1p = nc.declare_dram_parameter("b1", [1, HID], BF16, isOutput=False)
    b2p = nc.declare_dram_parameter("b2", [1, C], BF16, isOutput=False)
    ln1g = nc.declare_dram_parameter("ln1g", [C], F32, isOutput=False)
    ln1b = nc.declare_dram_parameter("ln1b", [C], F32, isOutput=False)
    ln2g = nc.declare_dram_parameter("ln2g", [C], F32, isOutput=False)
    ln2b = nc.declare_dram_parameter("ln2b", [C], F32, isOutput=False)
    colw = nc.declare_dram_parameter("colw", [NCH * 128, CHUNK_T * 8], I16, isOutput=False)
    biasP = nc.declare_dram_parameter("biasP", [NCH * 128, CHUNK_T * H], BF16, isOutput=False)
    ohP = nc.declare_dram_parameter("ohP", [NCH * 128, CHUNK_T * 128], BF16, isOutput=False)
    ohTP = nc.declare_dram_parameter("ohTP", [NCH * 128, CHUNK_T * 128], BF16, isOutput=False)
    y_out = nc.declare_dram_parameter("y", [LSH, C], F32, isOutput=True)

    with ExitStack() as ctx:
        tc = ctx.enter_context(tile.TileContext(nc))

        dram = ctx.enter_context(tc.tile_pool(name="dram", bufs=1, space="DRAM"))
        kv_sh = dram.tile([LSH, ROWB], FP8)
        kv_full = dram.tile([NCORES * LSH, ROWB], FP8)
        x1_d = dram.tile([LSH, C], F32)

        # ---------------- constants + weights ----------------
        consts = ctx.enter_context(tc.tile_pool(name="consts", bufs=1))
        ident = consts.tile([128, 128], BF16, tag="ident")
        make_identity(nc, ident[:])
        ones_k1 = consts.tile([1, 128], BF16, tag="ones")
        nc.vector.memset(ones_k1[:], 1.0)
        eps_t = consts.tile([128, 1], F32, tag="eps")
        nc.vector.memset(eps_t[:], EPS)

        def bcast_load(param, tag):
            t = consts.tile([128, C], F32, tag=tag)
            ap = param[:]
            src = bass.AP(tensor=ap.tensor, offset=ap.offset,
                          ap=[[0, 128], [1, C]])
            nc.sync.dma_start(out=t[:], in_=src)
            return t

        g1_bc, b1_bc = bcast_load(ln1g, "g1"), bcast_load(ln1b, "b1")
        g2_bc, b2_bc = bcast_load(ln2g, "g2"), bcast_load(ln2b, "b2")

        wts = ctx.enter_context(tc.tile_pool(name="wts", bufs=1))

        def wload(p, shape, tag):
            t = wts.tile(shape, BF16, tag=tag)
            nc.sync.dma_start(out=t[:], in_=p[:])
            return t

        wq_sb = wload(wq, [128, 4, C], "wq"); wk_sb = wload(wk, [128, 4, C], "wk")
        wv_sb = wload(wv, [128, 4, C], "wv"); wo_sb = wload(wo, [128, 4, C], "wo")
        w1_sb = wload(w1, [128, 4, HID], "w1"); w2_sb = wload(w2, [128, 8, C], "w2")
        bq_sb = wload(bqp, [1, C], "bq"); bk_sb = wload(bkp, [1, C], "bk")
        bv_sb = wload(bvp, [1, C], "bv"); bo_sb = wload(bop, [1, C], "bo")
        b1_sb = wload(b1p, [1, HID], "bb1"); b2_sb = wload(b2p, [1, C], "bb2")

        # Q table lives in SBUF for the whole kernel (gathered via one-hot PE
        # matmuls per edge tile, never written to DRAM)
        qall_pool = ctx.enter_context(tc.tile_pool(name="qall", bufs=1))
        q_all = qall_pool.tile([128, NBLK, C], BF16)

        # ---------------- LN helper ----------------
        def layernorm(pool, lnpool, xb, g_bc, bb_bc):
            """returns bf16 [128, C] normalized tile"""
            stats = lnpool.tile([128, 6], F32, tag="stats")
            nc.vector.bn_stats(stats[:], xb[:])
            mv = lnpool.tile([128, 2], F32, tag="mv")
            nc.vector.bn_aggr(mv[:], stats[:])
            sd = lnpool.tile([128, 1], F32, tag="sd")
            nc.scalar.activation(sd[:], mv[:, 1:2], AF.Sqrt, bias=eps_t[:])
            rstd = lnpool.tile([128, 1], F32, tag="rstd")
            nc.vector.reciprocal(rstd[:], sd[:])
            z0 = pool.tile([128, C], F32, tag="ln_z0")
            nc.vector.tensor_scalar(z0[:], xb[:], mv[:, 0:1], rstd[:],
                                    op0=ALU.subtract, op1=ALU.mult)
            z1 = pool.tile([128, C], F32, tag="ln_z1")
            nc.vector.tensor_mul(z1[:], z0[:], g_bc[:])
            zb = pool.tile([128, C], BF16, tag="ln_out")
            nc.vector.tensor_add(zb[:], z1[:], bb_bc[:])
            return zb

        # ---------------- phase B+C: LN1, zT, QKV ----------------
        with ExitStack() as pctx:
            zT_pool = pctx.enter_context(tc.tile_pool(name="zT", bufs=1))
            zT = zT_pool.tile([128, 4, LSH], BF16)
            xp = pctx.enter_context(tc.tile_pool(name="xp", bufs=3))
            lnp = pctx.enter_context(tc.tile_pool(name="lnp", bufs=4))
            trp = pctx.enter_context(tc.tile_pool(name="trp", bufs=2, space="PSUM"))
            qkvp = pctx.enter_context(tc.tile_pool(name="qkvp", bufs=2, space="PSUM"))
            obp = pctx.enter_context(tc.tile_pool(name="obp", bufs=3))

            for ib in range(NBLK):
                sl = slice(ib * 128, (ib + 1) * 128)
                xb = xp.tile([128, C], F32, tag="xin")
                nc.sync.dma_start(out=xb[:], in_=x_c[sl, :])
                zb = layernorm(xp, lnp, xb, g1_bc, b1_bc)
                for g in range(4):
                    pt = trp.tile([128, 128], BF16)
                    nc.tensor.transpose(pt[:], zb[:, g * 128:(g + 1) * 128], ident[:])
                    nc.scalar.copy(zT[:, g, sl], pt[:])
                for w_sb, bias_sb, dst in (
                    (wk_sb, bk_sb, "k"),
                    (wv_sb, bv_sb, "v"),
                    (wq_sb, bq_sb, "q"),
                ):
                    ps = qkvp.tile([128, C], F32)
                    for g in range(4):
                        nc.tensor.matmul(ps[:], lhsT=zT[:, g, sl], rhs=w_sb[:, g, :],
                                         start=(g == 0), stop=False)
                    nc.tensor.matmul(ps[:], lhsT=ones_k1[:], rhs=bias_sb[:],
                                     start=False, stop=True)
                    if dst == "v":
                        ob = obp.tile([128, C], FP8, tag="ob_v")
                        nc.scalar.copy(ob[:], ps[:])
                        nc.sync.dma_start(out=kv_sh[sl, 2 * C:3 * C], in_=ob[:])
                    elif dst == "q":
                        nc.scalar.copy(q_all[:, ib, :], ps[:])
                    else:
                        ob = obp.tile([128, C], BF16, tag="ob_k")
                        nc.scalar.copy(ob[:], ps[:])
                        nc.sync.dma_start(
                            out=kv_sh[sl, 0:2 * C].bitcast(BF16), in_=ob[:])

                # pipelined AllGather: fire quarter q as soon as its K/V
                # block writes are emitted (overlaps remaining projections)
                if (ib + 1) % (NBLK // QSPLIT) == 0:
                    q = (ib + 1) // (NBLK // QSPLIT) - 1
                    nc.gpsimd.collective_compute(
                        "AllGather", ALU.bypass,
                        replica_groups=[list(range(NCORES))],
                        ins=[kv_sh[q * ROWS_Q:(q + 1) * ROWS_Q, :]],
                        outs=[kv_full[q * NCORES * ROWS_Q:
                                      (q + 1) * NCORES * ROWS_Q, :]],
                    )

        # ---------------- phase E: edges ----------------
        with ExitStack() as pctx:
            kvp = pctx.enter_context(tc.tile_pool(name="kvp", bufs=2))
            ohp = pctx.enter_context(tc.tile_pool(name="ohp", bufs=2))
            ohtp = pctx.enter_context(tc.tile_pool(name="ohtp", bufs=2))
            idxp = pctx.enter_context(tc.tile_pool(name="idxp", bufs=3))
            bp = pctx.enter_context(tc.tile_pool(name="bp", bufs=2))
            scp = pctx.enter_context(tc.tile_pool(name="scp", bufs=2))
            pcp = pctx.enter_context(tc.tile_pool(name="pcp", bufs=2))
            qsb = pctx.enter_context(tc.tile_pool(name="qsb", bufs=2))
            wkp = pctx.enter_context(tc.tile_pool(name="wkp", bufs=2))
            wtp = pctx.enter_context(tc.tile_pool(name="wtp", bufs=3))
            vup_p = pctx.enter_context(tc.tile_pool(name="vup", bufs=2))
            finp = pctx.enter_context(tc.tile_pool(name="finp", bufs=2))
            qgps = pctx.enter_context(tc.tile_pool(name="qgps", bufs=2, space="PSUM"))
            pop_ = pctx.enter_context(tc.tile_pool(name="pout", bufs=2, space="PSUM"))
            psp = pctx.enter_context(tc.tile_pool(name="pssum", bufs=1, space="PSUM"))
            opp = pctx.enter_context(tc.tile_pool(name="opsum", bufs=1, space="PSUM"))

            pout = pssum = None
            for ch in range(NCH):
                tiles_c = min(CHUNK_T, NT - ch * CHUNK_T)
                n_idx = tiles_c * 128
                cidx = idxp.tile([128, CHUNK_T * 8], I16, tag="cidx")
                nc.sync.dma_start(out=cidx[:], in_=colw[ch * 128:(ch + 1) * 128, :])
                kvb = kvp.tile([128, CHUNK_T, ROWB], FP8)
                nc.gpsimd.dma_gather(
                    out_ap=kvb[:, :tiles_c, :], in_ap=kv_full[:],
                    idxs_ap=cidx[:, :n_idx // 16],
                    num_idxs=n_idx, num_idxs_reg=n_idx, elem_size=ROWB,
                    single_packet=False)
                ohc = ohp.tile([128, CHUNK_T, 128], BF16)
                nc.sync.dma_start(
                    out=ohc[:],
                    in_=ohP[ch * 128:(ch + 1) * 128, :]
                    .rearrange("p (t r) -> p t r", t=CHUNK_T))
                ohtc = ohtp.tile([128, CHUNK_T, 128], BF16)
                nc.sync.dma_start(
                    out=ohtc[:],
                    in_=ohTP[ch * 128:(ch + 1) * 128, :]
                    .rearrange("p (t r) -> p t r", t=CHUNK_T))
                bia = bp.tile([128, CHUNK_T, H], BF16)
                nc.sync.dma_start(
                    out=bia[:],
                    in_=biasP[ch * 128:(ch + 1) * 128, :]
                    .rearrange("p (t h) -> p t h", t=CHUNK_T))

                # bf16 view of the K region of the gathered rows:
                # [128, CHUNK_T, ROWB/2 bf16]; K = cols 0:C of each row
                kvKb = kvb[:].bitcast(BF16)  # [128, CHUNK_T, ROWB//2 bf16]

                # pass 1: per-tile Q one-hot gather (PE) + pair-batched
                # mul / fold / reduce / bias-add scores (vector)
                scc = scp.tile([128, CHUNK_T, H], F32)
                for s2 in range((tiles_c + 1) // 2):
                    s0 = s2 * 2
                    ns = min(2, tiles_c - s0)
                    qg2 = qgps.tile([128, 2, C], F32)
                    for j in range(ns):
                        slot = s0 + j
                        rb = tmap[ch * CHUNK_T + slot][0]
                        nc.tensor.matmul(qg2[:, j, :], lhsT=ohtc[:, slot, :],
                                         rhs=q_all[:, rb, :], start=True, stop=True)
                    qg_sb = qsb.tile([128, 2, C], BF16)
                    nc.scalar.copy(qg_sb[:, :ns, :], qg2[:, :ns, :])
                    prod = wkp.tile([128, 2, C], BF16, tag="prod")
                    nc.vector.tensor_tensor(
                        prod[:, :ns, :], kvKb[:, s0:s0 + ns, 0:C],
                        qg_sb[:, :ns, :], op=ALU.mult)
                    prodf = wkp.tile([128, 2, H, D // 2], BF16, tag="prodf")
                    pr4 = prod[:].rearrange("p t (h d) -> p t h d", h=H)
                    nc.vector.tensor_tensor(
                        prodf[:, :ns], pr4[:, :ns, :, 0:D // 2],
                        pr4[:, :ns, :, D // 2:D], op=ALU.add)
                    prodg = wkp.tile([128, 2, H, D // 4], BF16, tag="prodg")
                    nc.vector.tensor_tensor(
                        prodg[:, :ns], prodf[:, :ns, :, 0:D // 4],
                        prodf[:, :ns, :, D // 4:D // 2], op=ALU.add)
                    nc.vector.tensor_reduce(
                        scc[:, s0:s0 + ns, :], prodg[:, :ns],
                        axis=AX.X, op=ALU.add)
                    nc.vector.tensor_add(scc[:, s0:s0 + ns, :],
                                         scc[:, s0:s0 + ns, :],
                                         bia[:, s0:s0 + ns, :])
                pc = pcp.tile([128, CHUNK_T, H], BF16)
                nc.scalar.activation(pc[:, :tiles_c, :], scc[:, :tiles_c, :], AF.Exp)

                # pass 2: alpha-weighted scatter (one-hot PE matmuls).
                # gpsimd is kept gather-only; wt runs on vector at 2x with
                # scalar-engine upconvert (V fp8->bf16) + p expansion
                for s2 in range((tiles_c + 1) // 2):
                    s0 = s2 * 2
                    ns = min(2, tiles_c - s0)
                    vg2 = kvb[:, s0:s0 + ns, 2 * C:3 * C]
                    wt2 = wtp.tile([128, 2, C], BF16, tag="wt")
                    vup = vup_p.tile([128, 2, C], BF16, tag="vup")
                    nc.scalar.copy(vup[:, :ns, :], vg2)
                    pex = vup_p.tile([128, 2, C], BF16, tag="pex")
                    nc.scalar.copy(
                        pex[:, :ns, :].rearrange("p t (h d) -> p t h d", h=H),
                        _bc(pc[:, s0:s0 + ns, :], D))
                    nc.vector.tensor_tensor(wt2[:, :ns, :], vup[:, :ns, :],
                                            pex[:, :ns, :], op=ALU.mult)
                    for j in range(ns):
                        slot = s0 + j
                        t = ch * CHUNK_T + slot
                        rb, first, last = tmap[t]
                        if first:
                            pout = pop_.tile([128, C], F32)
                            pssum = psp.tile([128, H], F32)
                        nc.tensor.matmul(pout[:], lhsT=ohc[:, slot, :],
                                         rhs=wt2[:, j, :],
                                         start=first, stop=last)
                        nc.tensor.matmul(pssum[:], lhsT=ohc[:, slot, :],
                                         rhs=pc[:, slot, :],
                                         start=first, stop=last)
                        if not last:
                            continue
                        sl = slice(rb * 128, (rb + 1) * 128)
                        sm = finp.tile([128, H], F32, tag="sm")
                        nc.vector.tensor_scalar(sm[:], pssum[:], 1e-30, None, op0=ALU.max)
                        rec = finp.tile([128, H], F32, tag="rec")
                        nc.vector.reciprocal(rec[:], sm[:])
                        att = finp.tile([128, C], BF16, tag="att")
                        nc.vector.tensor_tensor(_phd(att[:]), _phd(pout[:]),
                                                _bc(rec[:], D), op=ALU.mult)
                        attT = finp.tile([128, 4, 128], BF16, tag="attT")
                        for g in range(4):
                            nc.sync.dma_start_transpose(
                                attT[:, g, :], att[:, g * 128:(g + 1) * 128])
                        po = opp.tile([128, C], F32)
                        for g in range(4):
                            nc.tensor.matmul(po[:], lhsT=attT[:, g, :], rhs=wo_sb[:, g, :],
                                             start=(g == 0), stop=False)
                        nc.tensor.matmul(po[:], lhsT=ones_k1[:], rhs=bo_sb[:],
                                         start=False, stop=True)
                        xb2 = finp.tile([128, C], F32, tag="xb2")
                        nc.sync.dma_start(out=xb2[:], in_=x_c[sl, :])
                        x1t = finp.tile([128, C], F32, tag="x1t")
                        nc.vector.tensor_add(x1t[:], po[:], xb2[:])
                        nc.sync.dma_start(out=x1_d[sl, :], in_=x1t[:])

        # ---------------- phase F: LN2 + MLP ----------------
        with ExitStack() as pctx:
            xp = pctx.enter_context(tc.tile_pool(name="xp2", bufs=3))
            lnp = pctx.enter_context(tc.tile_pool(name="lnp2", bufs=4))
            trp3 = pctx.enter_context(tc.tile_pool(name="trp3", bufs=2, space="PSUM"))
            hp = pctx.enter_context(tc.tile_pool(name="hpsum", bufs=1, space="PSUM"))
            yp = pctx.enter_context(tc.tile_pool(name="ypsum", bufs=1, space="PSUM"))
            sbp = pctx.enter_context(tc.tile_pool(name="sbp", bufs=3))

            for ib in range(NBLK):
                sl = slice(ib * 128, (ib + 1) * 128)
                x1t = xp.tile([128, C], F32, tag="x1in")
                nc.sync.dma_start(out=x1t[:], in_=x1_d[sl, :])
                z2 = layernorm(xp, lnp, x1t, g2_bc, b2_bc)
                z2T = sbp.tile([128, 4, 128], BF16, tag="z2T")
                for g in range(4):
                    pt = trp3.tile([128, 128], BF16)
                    nc.tensor.transpose(pt[:], z2[:, g * 128:(g + 1) * 128], ident[:])
                    nc.scalar.copy(z2T[:, g, :], pt[:])
                ph = hp.tile([128, 8, 128], F32)
                for chc in range(8):
                    csl = slice(chc * 128, (chc + 1) * 128)
                    for g in range(4):
                        nc.tensor.matmul(ph[:, chc, :], lhsT=w1_sb[:, g, csl],
                                         rhs=z2T[:, g, :], start=(g == 0), stop=False)
                    nc.tensor.matmul(ph[:, chc, :], lhsT=b1_sb[:, csl],
                                     rhs=ones_k1[:], start=False, stop=True)
                hs = sbp.tile([128, 8, 128], BF16, tag="hs")
                nc.scalar.activation(hs[:], ph[:], AF.Silu)
                py = yp.tile([128, C], F32)
                for chc in range(8):
                    nc.tensor.matmul(py[:], lhsT=hs[:, chc, :], rhs=w2_sb[:, chc, :],
                                     start=(chc == 0), stop=False)
                nc.tensor.matmul(py[:], lhsT=ones_k1[:], rhs=b2_sb[:],
                                 start=False, stop=True)
                yt = sbp.tile([128, C], F32, tag="yt")
                nc.vector.tensor_add(yt[:], py[:], x1t[:])
                nc.sync.dma_start(out=y_out[sl, :], in_=yt[:])

    nc.finalize()
    _split_multi_waits(nc)
    return nc


# --------------------------------------------------------------------------
# entry point
# --------------------------------------------------------------------------

def kernel(**inputs) -> np.ndarray:
    x = np.asarray(inputs["x"], np.float32)
    row = np.asarray(inputs["row_index"]).astype(np.int64)
    col = np.asarray(inputs["col_index"]).astype(np.int64)
    att_bias = np.asarray(inputs["att_bias"], np.float32)
    L = x.shape[0]
    LSH = L // NCORES

    tmap, NT, NCH, cores = _preprocess_edges(L, row, col, att_bias)
    w = _prep_weights(inputs)

    key = (L, tmap, NT, NCH)
    if key not in _prog_cache:
        _prog_cache[key] = _build_program(L, tmap, NT, NCH)
    nc = _prog_cache[key]

    in_maps = []
    for c in range(NCORES):
        m = dict(w)
        m["x_c"] = np.ascontiguousarray(x[c * LSH:(c + 1) * LSH])
        m.update(cores[c])
        in_maps.append(m)

    global LAST_EXEC_NS, LAST_RESULTS
    res = run_bass_kernel_spmd(nc, in_maps, list(range(NCORES)), trace=TRACE)
    LAST_RESULTS = res
    LAST_EXEC_NS = res.exec_time_ns
    return np.concatenate([res.results[c]["y"] for c in range(NCORES)], axis=0)

